# revision 15
# baseline (speedup 1.0000x reference)
"""Trainium2 Bass kernel for nn_MultiHeadAttention (B=4, T=2048, D=2048, H=16).

Sharding: tensor-parallel over heads. Each of 8 NeuronCores owns 2 heads
(256 of the 2048 Q/K/V dims). All matmul data is bf16 (1 cycle/row on the
PE at any width; rel-err budget 2e-2 leaves plenty of room).

Per core:
  phase 1: qT/kT projections in transposed layout [head_dim, tokens] and v
           in [tokens, head_dim] layout, streaming host-pretiled x-tiles.
           q/k/v stay RESIDENT in SBUF (bf16, 12MB); k/v also DMA out as
           bf16 external outputs (host casts to f32).
  phase 2: per (batch, head): causal attention at 128-row k-block
           granularity. Scores matmuls write PSUM groups of 2 k-blocks;
           one big exp activation per group (amortizes the ACT engine's
           352-cycle instruction overhead); 128x128 triangle mask on
           diagonal blocks only; denominator accumulated per-partition by
           the DVE and partition-reduced by one ones-matmul per q-chunk;
           AV accumulates into PSUM, then ctx = ctx_ps * recip(den) (DVE)
           into a resident bf16 ctx buffer.
  phase 3: out_partial[tok, :] = sum_h ctx_h.T @ WoT_h, written bf16.
Host: partials summed across cores in f32; k/v slices concatenated.
"""

import os
import sys

import numpy as np

for _p in ("/opt/trn_rl_repo",):
    if _p not in sys.path and os.path.isdir(_p):
        sys.path.insert(0, _p)

B, T, D, H = 4, 2048, 2048, 16
HD = 128
N_CORES = 8
HPC = H // N_CORES          # heads per core
DPC = HPC * HD              # q/k/v dims per core
NTOK = B * T

P = 128
QT = 512                    # q-chunk width (PSUM bank = 512 f32)
KC = 128                    # k-block granularity (= partition dim)
PT = 512                    # phase-1 token tile
DK = D // P                 # 16 contraction chunks
NPT = NTOK // PT            # 16 phase-1 tiles
NQC = T // QT               # 4 q-chunks per batch
TBLK = NTOK // P            # 64 phase-3 token blocks
GJ = 2                      # k-blocks per exp group

_CACHE = {}


def _build_module(use_ldw=False):
    import concourse.bass as bass  # noqa: F401
    import concourse.mybir as mybir
    from concourse import bacc
    import concourse.tile as tile

    F32 = mybir.dt.float32
    F32R = mybir.dt.float32r
    BF16 = mybir.dt.bfloat16
    AF = mybir.ActivationFunctionType

    SCALE = 1.0 / float(np.sqrt(HD))

    nc = bacc.Bacc("TRN2", target_bir_lowering=False, debug=False)

    # host-pretiled inputs (all bf16, partition-major contiguous)
    xt = nc.dram_tensor("xt", [NPT, P, DK, PT], BF16, kind="ExternalInput").ap()
    wq = nc.dram_tensor("wq", [P, DK, DPC], BF16, kind="ExternalInput").ap()
    wk = nc.dram_tensor("wk", [P, DK, DPC], BF16, kind="ExternalInput").ap()
    wv = nc.dram_tensor("wv", [P, DK, DPC], BF16, kind="ExternalInput").ap()
    wo = nc.dram_tensor("wo", [P, HPC, D], BF16, kind="ExternalInput").ap()
    tri = nc.dram_tensor("tri", [P, P], BF16, kind="ExternalInput").ap()

    kT_out = nc.dram_tensor("kT_out", [DPC, NTOK], BF16, kind="ExternalOutput").ap()
    v_out = nc.dram_tensor("v_out", [NTOK, DPC], BF16, kind="ExternalOutput").ap()
    out_p = nc.dram_tensor("out_p", [NTOK, D], BF16, kind="ExternalOutput").ap()

    def mm(out, lhsT, rhs, **kw):
        if use_ldw and lhsT.dtype not in (F32, F32R):
            nc.tensor.ldweights(lhsT)
        nc.tensor.matmul(out, lhsT, rhs, **kw)

    with tile.TileContext(nc) as tc:
        with (
            tc.tile_pool(name="res", bufs=1) as res_pool,
            tc.tile_pool(name="cst", bufs=1) as cst_pool,
        ):
            q_res = res_pool.tile([P, HPC, NTOK], BF16, tag="q")
            k_res = res_pool.tile([P, HPC, NTOK], BF16, tag="k")
            v_res = res_pool.tile([P, TBLK, DPC], BF16, tag="v")
            ctx_res = res_pool.tile([P, HPC, NTOK], BF16, tag="ctx")
            tri_sb = cst_pool.tile([P, P], BF16, tag="tri")
            ones_f = cst_pool.tile([P, P], F32, tag="onesf")
            ones_sb = cst_pool.tile([P, P], BF16, tag="ones")
            nc.sync.dma_start(tri_sb[:], tri)
            nc.vector.memset(ones_f[:], 1.0)
            nc.vector.tensor_copy(ones_sb[:], ones_f[:])

            # ---------------- Phase 1: projections ----------------
            with (
                tc.tile_pool(name="wgt", bufs=1) as w_pool,
                tc.tile_pool(name="xt", bufs=2) as xt_pool,
                tc.tile_pool(name="pp_qk", bufs=2, space="PSUM") as pp_qk,
                tc.tile_pool(name="pp_v", bufs=2, space="PSUM") as pp_v,
            ):
                wq_sb = w_pool.tile([P, DK, DPC], BF16, tag="wq")
                wk_sb = w_pool.tile([P, DK, DPC], BF16, tag="wk")
                wv_sb = w_pool.tile([P, DK, DPC], BF16, tag="wv")
                nc.sync.dma_start(wq_sb[:], wq)

                def load_xt(tb):
                    x_sb = xt_pool.tile([P, DK, PT], BF16, tag="x")
                    nc.sync.dma_start(x_sb[:], xt[tb])
                    return x_sb

                x0 = load_xt(0)
                nc.sync.dma_start(wk_sb[:], wk)
                nc.sync.dma_start(wv_sb[:], wv)

                for tb in range(NPT):
                    x_sb = x0 if tb == 0 else load_xt(tb)
                    ts = slice(tb * PT, (tb + 1) * PT)

                    for w_sb, dst in ((wq_sb, q_res), (wk_sb, k_res)):
                        for hc in range(HPC):
                            ps = pp_qk.tile([P, PT], F32, tag="pqk")
                            for dc in range(DK):
                                mm(ps[:],
                                   w_sb[:, dc, hc * P:(hc + 1) * P],
                                   x_sb[:, dc, :],
                                   start=(dc == 0), stop=(dc == DK - 1))
                            nc.vector.tensor_copy(dst[:, hc, ts], ps[:])
                            if dst is k_res:
                                nc.sync.dma_start(
                                    kT_out[hc * P:(hc + 1) * P, ts],
                                    k_res[:, hc, ts])

                    for sub in range(PT // P):
                        blk = tb * (PT // P) + sub
                        ps = pp_v.tile([P, DPC], F32, tag="pv")
                        for dc in range(DK):
                            mm(ps[:],
                               x_sb[:, dc, sub * P:(sub + 1) * P],
                               wv_sb[:, dc, :],
                               start=(dc == 0), stop=(dc == DK - 1))
                        nc.vector.tensor_copy(v_res[:, blk, :], ps[:])
                        nc.sync.dma_start(
                            v_out[blk * P:(blk + 1) * P, :], v_res[:, blk, :])

            # ---------------- Phase 2: attention ----------------
            with (
                tc.tile_pool(name="exp", bufs=4) as exp_pool,
                tc.tile_pool(name="rcp", bufs=2) as rcp_pool,
                tc.tile_pool(name="pp_s", bufs=2, space="PSUM") as pp_s,
                tc.tile_pool(name="pp_ctx", bufs=2, space="PSUM") as pp_ctx,
                tc.tile_pool(name="pp_den", bufs=2, space="PSUM") as pp_den,
            ):
                def do_pair(b, h):
                    boff = b * T
                    qv = q_res[:, h, boff:boff + T]
                    kv = k_res[:, h, boff:boff + T]
                    for c in range(NQC):
                        nj = (c + 1) * (QT // KC)       # active k-blocks
                        ngrp = nj // GJ
                        ctx_ps = pp_ctx.tile([P, QT], F32, tag="pctx")
                        den_ps = pp_den.tile([P, QT], F32, tag="pden")
                        q0 = c * QT
                        for g in range(ngrp):
                            grp_ps = pp_s.tile([P, GJ * QT], F32, tag="ps")
                            e_grp = exp_pool.tile([P, GJ * QT], BF16, tag="e")
                            offs = []
                            for s in range(GJ):
                                j = g * GJ + s
                                o = max(0, (j - 4 * c) * KC)
                                offs.append(o)
                                mm(grp_ps[:, s * QT + o:(s + 1) * QT],
                                   kv[:, j * KC:(j + 1) * KC],
                                   qv[:, q0 + o:q0 + QT],
                                   start=True, stop=True)
                            # exp over each contiguous written run (pads in
                            # diagonal groups are never written nor read)
                            runs = []
                            for s in range(GJ):
                                lo = s * QT + offs[s]
                                hi = (s + 1) * QT
                                if runs and runs[-1][1] == lo:
                                    runs[-1][1] = hi
                                else:
                                    runs.append([lo, hi])
                            for lo, hi in runs:
                                nc.scalar.activation(
                                    e_grp[:, lo:hi], grp_ps[:, lo:hi],
                                    AF.Exp, scale=SCALE)
                            for s in range(GJ):
                                j = g * GJ + s
                                o = offs[s]
                                if j >= 4 * c:      # diagonal: triangle mask
                                    nc.vector.tensor_mul(
                                        e_grp[:, s * QT + o:s * QT + o + P],
                                        e_grp[:, s * QT + o:s * QT + o + P],
                                        tri_sb[:])
                            for s in range(GJ):
                                j = g * GJ + s
                                o = offs[s]
                                esub = e_grp[:, s * QT + o:(s + 1) * QT]
                                mm(ctx_ps[:, o:],
                                   v_res[:, b * (T // P) + j,
                                         h * HD:(h + 1) * HD],
                                   esub,
                                   start=(j == 0), stop=(j == nj - 1),
                                   skip_group_check=True)
                                # denominator: ones-matmul partition-reduce,
                                # accumulated in PSUM across k-blocks
                                mm(den_ps[:, o:], ones_sb[:], esub,
                                   start=(j == 0), stop=(j == nj - 1),
                                   skip_group_check=True)
                        recip = rcp_pool.tile([P, QT], F32, tag="rcp")
                        nc.vector.reciprocal(recip[:], den_ps[:])
                        nc.vector.tensor_mul(
                            ctx_res[:, h, boff + q0:boff + q0 + QT],
                            ctx_ps[:], recip[:])

                for b in range(B):
                    for h in range(HPC):
                        do_pair(b, h)

            # ---------------- Phase 3: output projection ----------------
            with (
                tc.tile_pool(name="wo", bufs=1) as wo_pool,
                tc.tile_pool(name="st3", bufs=3) as st3_pool,
                tc.tile_pool(name="pp_o", bufs=2, space="PSUM") as pp_o,
            ):
                wo_sb = wo_pool.tile([P, HPC, D], BF16, tag="wo")
                nc.sync.dma_start(wo_sb[:], wo)
                NOD = D // QT
                for tb in range(TBLK):
                    ts2 = slice(tb * P, (tb + 1) * P)
                    ost = st3_pool.tile([P, D], BF16, tag="ost")
                    for od in range(NOD):
                        ods = slice(od * QT, (od + 1) * QT)
                        ps0 = pp_o.tile([P, QT], F32, tag="po")
                        mm(ps0[:], ctx_res[:, 0, ts2], wo_sb[:, 0, ods],
                           start=True, stop=False)
                        mm(ps0[:], ctx_res[:, 1, ts2], wo_sb[:, 1, ods],
                           start=False, stop=True)
                        if od % 2 == 0:
                            nc.vector.tensor_copy(ost[:, ods], ps0[:])
                        else:
                            nc.scalar.copy(ost[:, ods], ps0[:])
                    nc.sync.dma_start(out_p[ts2, :], ost[:])

    nc.compile()
    return nc


def _build_fused():
    """Software-pipelined variant: phase-1 projections of batch b+1 and
    phase-3 output blocks are interleaved (at emission level) with the
    attention of batch b, so the PE never idles on exp/copy latency."""
    import concourse.bass as bass  # noqa: F401
    import concourse.mybir as mybir
    from concourse import bacc
    import concourse.tile as tile

    F32 = mybir.dt.float32
    BF16 = mybir.dt.bfloat16
    AF = mybir.ActivationFunctionType

    SCALE = 1.0 / float(np.sqrt(HD))
    TPB = T // PT               # phase-1 tiles per batch (4)

    nc = bacc.Bacc("TRN2", target_bir_lowering=False, debug=False)

    xt = nc.dram_tensor("xt", [NPT, P, DK, PT], BF16, kind="ExternalInput").ap()
    wq = nc.dram_tensor("wq", [HPC, P, DK, P], BF16, kind="ExternalInput").ap()
    wk = nc.dram_tensor("wk", [HPC, P, DK, P], BF16, kind="ExternalInput").ap()
    wv = nc.dram_tensor("wv", [HPC, P, DK, P], BF16, kind="ExternalInput").ap()
    wo = nc.dram_tensor("wo", [P, HPC, D], BF16, kind="ExternalInput").ap()
    tri = nc.dram_tensor("tri", [P, P], BF16, kind="ExternalInput").ap()

    kT_out = nc.dram_tensor("kT_out", [DPC, NTOK], BF16, kind="ExternalOutput").ap()
    v_out = nc.dram_tensor("v_out", [NTOK, DPC], BF16, kind="ExternalOutput").ap()
    out_p = nc.dram_tensor("out_p", [NTOK, D], BF16, kind="ExternalOutput").ap()

    mm = nc.tensor.matmul

    def interleave(*gens):
        """Round-robin emission from generators, proportional to their
        remaining unit counts (gen, n_units) pairs."""
        live = [[g, n] for g, n in gens if n > 0]
        credit = [0.0] * len(live)
        while live:
            for i, it in enumerate(live):
                credit[i] += it[1]
            i = max(range(len(live)), key=lambda i: credit[i])
            credit[i] -= sum(it[1] for it in live)
            try:
                next(live[i][0])
            except StopIteration:
                credit.pop(i)
                live.pop(i)

    with tile.TileContext(nc) as tc:
        with (
            tc.tile_pool(name="res", bufs=1) as res_pool,
            tc.tile_pool(name="cst", bufs=1) as cst_pool,
        ):
            q_res = res_pool.tile([P, HPC, NTOK], BF16, tag="q")
            k_res = res_pool.tile([P, HPC, NTOK], BF16, tag="k")
            v_res = res_pool.tile([P, TBLK, DPC], BF16, tag="v")
            ctx_res = res_pool.tile([P, HPC, NTOK], BF16, tag="ctx")
            tri_sb = cst_pool.tile([P, P], BF16, tag="tri")
            ones_f = cst_pool.tile([P, P], F32, tag="onesf")
            ones_sb = cst_pool.tile([P, P], BF16, tag="ones")
            wo_sb = cst_pool.tile([P, HPC, D], BF16, tag="wo")
            nc.sync.dma_start(tri_sb[:], tri)
            nc.vector.memset(ones_f[:], 1.0)
            nc.vector.tensor_copy(ones_sb[:], ones_f[:])

            with (
                tc.tile_pool(name="exp", bufs=3) as exp_pool,
                tc.tile_pool(name="rcp", bufs=1) as rcp_pool,
                tc.tile_pool(name="pp_s", bufs=2, space="PSUM") as pp_s,
                tc.tile_pool(name="pp_ctx", bufs=1, space="PSUM") as pp_ctx,
                tc.tile_pool(name="pp_den", bufs=1, space="PSUM") as pp_den,
            ):
                def p2_chunk(b, h, c):
                    """One yield per exp-group (2 k-blocks)."""
                    boff = b * T
                    qv = q_res[:, h, boff:boff + T]
                    kv = k_res[:, h, boff:boff + T]
                    if True:
                        nj = (c + 1) * (QT // KC)
                        ngrp = nj // GJ
                        ctx_ps = pp_ctx.tile([P, QT], F32, tag="pctx")
                        den_ps = pp_den.tile([P, QT], F32, tag="pden")
                        q0 = c * QT
                        for g in range(ngrp):
                            grp_ps = pp_s.tile([P, GJ * QT], F32, tag="ps")
                            e_grp = exp_pool.tile([P, GJ * QT], BF16, tag="e")
                            offs = []
                            for s in range(GJ):
                                j = g * GJ + s
                                o = max(0, (j - 4 * c) * KC)
                                offs.append(o)
                                mm(grp_ps[:, s * QT + o:(s + 1) * QT],
                                   kv[:, j * KC:(j + 1) * KC],
                                   qv[:, q0 + o:q0 + QT],
                                   start=True, stop=True)
                            runs = []
                            for s in range(GJ):
                                lo = s * QT + offs[s]
                                hi = (s + 1) * QT
                                if runs and runs[-1][1] == lo:
                                    runs[-1][1] = hi
                                else:
                                    runs.append([lo, hi])
                            for lo, hi in runs:
                                nc.scalar.activation(
                                    e_grp[:, lo:hi], grp_ps[:, lo:hi],
                                    AF.Exp, scale=SCALE)
                            for s in range(GJ):
                                j = g * GJ + s
                                o = offs[s]
                                if j >= 4 * c:
                                    nc.vector.tensor_mul(
                                        e_grp[:, s * QT + o:s * QT + o + P],
                                        e_grp[:, s * QT + o:s * QT + o + P],
                                        tri_sb[:])
                            for s in range(GJ):
                                j = g * GJ + s
                                o = offs[s]
                                esub = e_grp[:, s * QT + o:(s + 1) * QT]
                                mm(ctx_ps[:, o:],
                                   v_res[:, b * (T // P) + j,
                                         h * HD:(h + 1) * HD],
                                   esub,
                                   start=(j == 0), stop=(j == nj - 1),
                                   skip_group_check=True)
                                mm(den_ps[:, o:], ones_sb[:], esub,
                                   start=(j == 0), stop=(j == nj - 1),
                                   skip_group_check=True)
                            if g == ngrp - 1:
                                recip = rcp_pool.tile([P, QT], F32, tag="rcp")
                                nc.vector.reciprocal(recip[:], den_ps[:])
                                nc.vector.tensor_mul(
                                    ctx_res[:, h, boff + q0:boff + q0 + QT],
                                    ctx_ps[:], recip[:])
                            yield

                def p2_batch(b):
                    for h in range(HPC):
                        for c in range(NQC):
                            yield from p2_chunk(b, h, c)

                # ---- segment A: p1(b) interleaved with p2(b-1) ----
                with (
                    tc.tile_pool(name="wgt", bufs=1) as w_pool,
                    tc.tile_pool(name="xt", bufs=2) as xt_pool,
                    tc.tile_pool(name="pp_1", bufs=1, space="PSUM") as pp_1,
                ):
                    wq_sb = w_pool.tile([P, DK, DPC], BF16, tag="wq")
                    wk_sb = w_pool.tile([P, DK, DPC], BF16, tag="wk")
                    wv_sb = w_pool.tile([P, DK, DPC], BF16, tag="wv")
                    # split so the hc=0 chains start as soon as possible
                    for hc in range(HPC):
                        nc.sync.dma_start(
                            wq_sb[:, :, hc * P:(hc + 1) * P], wq[hc])

                    def load_xt(tb, split=False):
                        x_sb = xt_pool.tile([P, DK, PT], BF16, tag="x")
                        if split:
                            h1 = DK // 2
                            nc.sync.dma_start(x_sb[:, :h1, :], xt[tb, :, :h1, :])
                            nc.sync.dma_start(x_sb[:, h1:, :], xt[tb, :, h1:, :])
                        else:
                            nc.sync.dma_start(x_sb[:], xt[tb])
                        return x_sb

                    x0 = load_xt(0, split=True)
                    for hc in range(HPC):
                        nc.sync.dma_start(
                            wk_sb[:, :, hc * P:(hc + 1) * P], wk[hc])
                    for hc in range(HPC):
                        nc.sync.dma_start(
                            wv_sb[:, :, hc * P:(hc + 1) * P], wv[hc])
                    nc.sync.dma_start(wo_sb[:], wo)

                    def p1_units(b):
                        """One yield per 16-matmul chain (8 per tile)."""
                        for tloc in range(TPB):
                            tb = b * TPB + tloc
                            x_sb = x0 if tb == 0 else load_xt(tb)
                            ts = slice(tb * PT, (tb + 1) * PT)
                            for w_sb, dst in ((wq_sb, q_res), (wk_sb, k_res)):
                                for hc in range(HPC):
                                    ps = pp_1.tile([P, PT], F32, tag="p1")
                                    for dc in range(DK):
                                        mm(ps[:],
                                           w_sb[:, dc, hc * P:(hc + 1) * P],
                                           x_sb[:, dc, :],
                                           start=(dc == 0),
                                           stop=(dc == DK - 1))
                                    nc.vector.tensor_copy(dst[:, hc, ts], ps[:])
                                    if dst is k_res:
                                        nc.sync.dma_start(
                                            kT_out[hc * P:(hc + 1) * P, ts],
                                            k_res[:, hc, ts])
                                    yield
                            for sub in range(PT // P):
                                blk = tb * (PT // P) + sub
                                psf = pp_1.tile([P, PT], F32, tag="p1")
                                ps = psf[:, 0:DPC]
                                for dc in range(DK):
                                    mm(ps[:],
                                       x_sb[:, dc, sub * P:(sub + 1) * P],
                                       wv_sb[:, dc, :],
                                       start=(dc == 0), stop=(dc == DK - 1))
                                nc.vector.tensor_copy(v_res[:, blk, :], ps[:])
                                nc.sync.dma_start(
                                    v_out[blk * P:(blk + 1) * P, :],
                                    v_res[:, blk, :])
                                yield

                    # prologue + first fused segment (no p3 yet)
                    for _ in p1_units(0):
                        pass
                    interleave((p1_units(1), 32), (p2_batch(0), 40))

                    with (
                        tc.tile_pool(name="st3", bufs=3) as st3_pool,
                        tc.tile_pool(name="pp_o", bufs=1, space="PSUM") as pp_o,
                    ):
                        NOD = D // QT

                        def p3_units(blocks):
                            """One yield per token block (8 matmuls)."""
                            for tb in blocks:
                                ts2 = slice(tb * P, (tb + 1) * P)
                                for half in range(2):
                                    ost = st3_pool.tile([P, D // 2], BF16,
                                                        tag="ost")
                                    for odl in range(NOD // 2):
                                        od = half * (NOD // 2) + odl
                                        ods = slice(od * QT, (od + 1) * QT)
                                        odl_s = slice(odl * QT,
                                                      (odl + 1) * QT)
                                        ps0 = pp_o.tile([P, QT], F32,
                                                        tag="po")
                                        mm(ps0[:], ctx_res[:, 0, ts2],
                                           wo_sb[:, 0, ods],
                                           start=True, stop=False)
                                        mm(ps0[:], ctx_res[:, 1, ts2],
                                           wo_sb[:, 1, ods],
                                           start=False, stop=True)
                                        nc.vector.tensor_copy(
                                            ost[:, odl_s], ps0[:])
                                    nc.sync.dma_start(
                                        out_p[ts2, half * (D // 2):
                                              (half + 1) * (D // 2)],
                                        ost[:])
                                yield

                        TB = T // P     # 16 token blocks per batch

                        def brange(b):
                            return range(b * TB, (b + 1) * TB)

                        # p1(b) || p2(b-1) || p3(b-2) for b = 2, 3
                        for b in (2, 3):
                            interleave((p1_units(b), 32),
                                       (p2_batch(b - 1), 40),
                                       (p3_units(brange(b - 2)), 16))

                        # tail: p2(3) chunk-major with p3(3) blocks inlined
                        # as their ctx becomes ready, p3(2) as filler
                        fill = p3_units(brange(2))

                        def p2_tail(b):
                            for c in range(NQC):
                                for h in range(HPC):
                                    yield from p2_chunk(b, h, c)
                                yield ("p3", c)

                        nfill = [16]
                        fcredit = [0.0]

                        def drain_fill(n):
                            fcredit[0] += n * (16.0 / 56.0)
                            while fcredit[0] >= 1.0 and nfill[0] > 0:
                                fcredit[0] -= 1.0
                                try:
                                    next(fill)
                                    nfill[0] -= 1
                                except StopIteration:
                                    nfill[0] = 0

                        b3 = B - 1
                        for unit in p2_tail(b3):
                            if isinstance(unit, tuple):
                                c = unit[1]
                                for _ in p3_units(
                                        range(b3 * TB + c * 4,
                                              b3 * TB + (c + 1) * 4)):
                                    drain_fill(1)
                            else:
                                drain_fill(1)
                        drain_fill(99)

    nc.compile()
    return nc


def _get_module():
    if "nc" not in _CACHE:
        if os.environ.get("BASS_FUSED", "1") == "1":
            _CACHE["nc"] = _build_fused()
        else:
            use_ldw = os.environ.get("BASS_USE_LDW", "0") == "1"
            _CACHE["nc"] = _build_module(use_ldw=use_ldw)
    return _CACHE["nc"]


def _make_tri():
    import ml_dtypes
    m = np.zeros((P, P), dtype=np.float32)
    for kk in range(P):
        m[kk, kk:] = 1.0
    return m.astype(ml_dtypes.bfloat16)


def _run(x, Wq, Wk, Wv, Wo, bo, trace=False):
    import ml_dtypes
    from concourse import bass_utils

    BF = ml_dtypes.bfloat16
    nc = _get_module()
    x = np.asarray(x, dtype=np.float32)
    # xt[tb, p, dk, t] = x[tb*PT + t, dk*P + p]
    xt = np.ascontiguousarray(
        x.reshape(NPT, PT, DK, P).transpose(0, 3, 2, 1)).astype(BF)
    tri = _make_tri()
    Wq = np.asarray(Wq, np.float32)
    Wk = np.asarray(Wk, np.float32)
    Wv = np.asarray(Wv, np.float32)
    Wo = np.asarray(Wo, np.float32)
    in_maps = []
    for c in range(N_CORES):
        sl = slice(c * DPC, (c + 1) * DPC)
        # w[p, dk, n] = W[c*DPC + n, dk*P + p]
        def wtile(W):
            # [hc, p, dk, d] with d = output dim within head
            return np.ascontiguousarray(
                W[sl, :].reshape(HPC, P, DK, P).transpose(0, 3, 2, 1)
            ).astype(BF)
        # wo[p, h, n] = Wo[n, c*DPC + h*P + p]
        wo_t = np.ascontiguousarray(
            Wo[:, sl].reshape(D, HPC, P).transpose(2, 1, 0)).astype(BF)
        in_maps.append({
            "xt": xt,
            "wq": wtile(Wq),
            "wk": wtile(Wk),
            "wv": wtile(Wv),
            "wo": wo_t,
            "tri": tri,
        })
    res = bass_utils.run_bass_kernel_spmd(
        nc, in_maps, core_ids=list(range(N_CORES)), trace=trace)

    out = np.zeros((NTOK, D), np.float32)
    k = np.empty((NTOK, D), np.float32)
    v = np.empty((NTOK, D), np.float32)
    for c, r in enumerate(res.results):
        sl = slice(c * DPC, (c + 1) * DPC)
        out += np.asarray(r["out_p"], dtype=np.float32)
        k[:, sl] = np.asarray(r["kT_out"], dtype=np.float32).T
        v[:, sl] = np.asarray(r["v_out"], dtype=np.float32)
    out += np.asarray(bo, np.float32)[None, :]
    outs = (out.reshape(B, T, D), k.reshape(B, T, D), v.reshape(B, T, D))
    return outs, res


def kernel(x, Wq, Wk, Wv, Wo, bo):
    outs, _ = _run(x, Wq, Wk, Wv, Wo, bo, trace=False)
    return outs


# revision 16
# speedup vs baseline: 1.3326x; 1.3326x over previous
"""Trainium2 Bass kernel for nn_MultiHeadAttention (B=4, T=2048, D=2048, H=16).

Sharding: tensor-parallel over heads. Each of 8 NeuronCores owns 2 heads
(256 of the 2048 Q/K/V dims). All matmul data is bf16 (1 cycle/row on the
PE at any width; rel-err budget 2e-2 leaves plenty of room).

Per core:
  phase 1: qT/kT projections in transposed layout [head_dim, tokens] and v
           in [tokens, head_dim] layout, streaming host-pretiled x-tiles.
           q/k/v stay RESIDENT in SBUF (bf16, 12MB); k/v also DMA out as
           bf16 external outputs (host casts to f32).
  phase 2: per (batch, head): causal attention at 128-row k-block
           granularity. Scores matmuls write PSUM groups of 2 k-blocks;
           one big exp activation per group (amortizes the ACT engine's
           352-cycle instruction overhead); 128x128 triangle mask on
           diagonal blocks only; denominator accumulated per-partition by
           the DVE and partition-reduced by one ones-matmul per q-chunk;
           AV accumulates into PSUM, then ctx = ctx_ps * recip(den) (DVE)
           into a resident bf16 ctx buffer.
  phase 3: out_partial[tok, :] = sum_h ctx_h.T @ WoT_h, written bf16.
Host: partials summed across cores in f32; k/v slices concatenated.
"""

import os
import sys

import numpy as np

for _p in ("/opt/trn_rl_repo",):
    if _p not in sys.path and os.path.isdir(_p):
        sys.path.insert(0, _p)

B, T, D, H = 4, 2048, 2048, 16
HD = 128
N_CORES = 8
HPC = H // N_CORES          # heads per core
DPC = HPC * HD              # q/k/v dims per core
NTOK = B * T

P = 128
QT = 512                    # q-chunk width (PSUM bank = 512 f32)
KC = 128                    # k-block granularity (= partition dim)
PT = 512                    # phase-1 token tile
DK = D // P                 # 16 contraction chunks
NPT = NTOK // PT            # 16 phase-1 tiles
NQC = T // QT               # 4 q-chunks per batch
TBLK = NTOK // P            # 64 phase-3 token blocks
GJ = 2                      # k-blocks per exp group

_CACHE = {}


def _build_module(use_ldw=False):
    import concourse.bass as bass  # noqa: F401
    import concourse.mybir as mybir
    from concourse import bacc
    import concourse.tile as tile

    F32 = mybir.dt.float32
    F32R = mybir.dt.float32r
    BF16 = mybir.dt.bfloat16
    AF = mybir.ActivationFunctionType

    SCALE = 1.0 / float(np.sqrt(HD))

    nc = bacc.Bacc("TRN2", target_bir_lowering=False, debug=False)

    # host-pretiled inputs (all bf16, partition-major contiguous)
    xt = nc.dram_tensor("xt", [NPT, P, DK, PT], BF16, kind="ExternalInput").ap()
    wq = nc.dram_tensor("wq", [P, DK, DPC], BF16, kind="ExternalInput").ap()
    wk = nc.dram_tensor("wk", [P, DK, DPC], BF16, kind="ExternalInput").ap()
    wv = nc.dram_tensor("wv", [P, DK, DPC], BF16, kind="ExternalInput").ap()
    wo = nc.dram_tensor("wo", [P, HPC, D], BF16, kind="ExternalInput").ap()
    tri = nc.dram_tensor("tri", [P, P], BF16, kind="ExternalInput").ap()

    kT_out = nc.dram_tensor("kT_out", [DPC, NTOK], BF16, kind="ExternalOutput").ap()
    v_out = nc.dram_tensor("v_out", [NTOK, DPC], BF16, kind="ExternalOutput").ap()
    out_p = nc.dram_tensor("out_p", [NTOK, D], BF16, kind="ExternalOutput").ap()

    def mm(out, lhsT, rhs, **kw):
        if use_ldw and lhsT.dtype not in (F32, F32R):
            nc.tensor.ldweights(lhsT)
        nc.tensor.matmul(out, lhsT, rhs, **kw)

    with tile.TileContext(nc) as tc:
        with (
            tc.tile_pool(name="res", bufs=1) as res_pool,
            tc.tile_pool(name="cst", bufs=1) as cst_pool,
        ):
            q_res = res_pool.tile([P, HPC, NTOK], BF16, tag="q")
            k_res = res_pool.tile([P, HPC, NTOK], BF16, tag="k")
            v_res = res_pool.tile([P, TBLK, DPC], BF16, tag="v")
            ctx_res = res_pool.tile([P, HPC, NTOK], BF16, tag="ctx")
            tri_sb = cst_pool.tile([P, P], BF16, tag="tri")
            ones_f = cst_pool.tile([P, P], F32, tag="onesf")
            ones_sb = cst_pool.tile([P, P], BF16, tag="ones")
            nc.sync.dma_start(tri_sb[:], tri)
            nc.vector.memset(ones_f[:], 1.0)
            nc.vector.tensor_copy(ones_sb[:], ones_f[:])

            # ---------------- Phase 1: projections ----------------
            with (
                tc.tile_pool(name="wgt", bufs=1) as w_pool,
                tc.tile_pool(name="xt", bufs=2) as xt_pool,
                tc.tile_pool(name="pp_qk", bufs=2, space="PSUM") as pp_qk,
                tc.tile_pool(name="pp_v", bufs=2, space="PSUM") as pp_v,
            ):
                wq_sb = w_pool.tile([P, DK, DPC], BF16, tag="wq")
                wk_sb = w_pool.tile([P, DK, DPC], BF16, tag="wk")
                wv_sb = w_pool.tile([P, DK, DPC], BF16, tag="wv")
                nc.sync.dma_start(wq_sb[:], wq)

                def load_xt(tb):
                    x_sb = xt_pool.tile([P, DK, PT], BF16, tag="x")
                    nc.sync.dma_start(x_sb[:], xt[tb])
                    return x_sb

                x0 = load_xt(0)
                nc.sync.dma_start(wk_sb[:], wk)
                nc.sync.dma_start(wv_sb[:], wv)

                for tb in range(NPT):
                    x_sb = x0 if tb == 0 else load_xt(tb)
                    ts = slice(tb * PT, (tb + 1) * PT)

                    for w_sb, dst in ((wq_sb, q_res), (wk_sb, k_res)):
                        for hc in range(HPC):
                            ps = pp_qk.tile([P, PT], F32, tag="pqk")
                            for dc in range(DK):
                                mm(ps[:],
                                   w_sb[:, dc, hc * P:(hc + 1) * P],
                                   x_sb[:, dc, :],
                                   start=(dc == 0), stop=(dc == DK - 1))
                            nc.vector.tensor_copy(dst[:, hc, ts], ps[:])
                            if dst is k_res:
                                nc.sync.dma_start(
                                    kT_out[hc * P:(hc + 1) * P, ts],
                                    k_res[:, hc, ts])

                    for sub in range(PT // P):
                        blk = tb * (PT // P) + sub
                        ps = pp_v.tile([P, DPC], F32, tag="pv")
                        for dc in range(DK):
                            mm(ps[:],
                               x_sb[:, dc, sub * P:(sub + 1) * P],
                               wv_sb[:, dc, :],
                               start=(dc == 0), stop=(dc == DK - 1))
                        nc.vector.tensor_copy(v_res[:, blk, :], ps[:])
                        nc.sync.dma_start(
                            v_out[blk * P:(blk + 1) * P, :], v_res[:, blk, :])

            # ---------------- Phase 2: attention ----------------
            with (
                tc.tile_pool(name="exp", bufs=4) as exp_pool,
                tc.tile_pool(name="rcp", bufs=2) as rcp_pool,
                tc.tile_pool(name="pp_s", bufs=2, space="PSUM") as pp_s,
                tc.tile_pool(name="pp_ctx", bufs=2, space="PSUM") as pp_ctx,
                tc.tile_pool(name="pp_den", bufs=2, space="PSUM") as pp_den,
            ):
                def do_pair(b, h):
                    boff = b * T
                    qv = q_res[:, h, boff:boff + T]
                    kv = k_res[:, h, boff:boff + T]
                    for c in range(NQC):
                        nj = (c + 1) * (QT // KC)       # active k-blocks
                        ngrp = nj // GJ
                        ctx_ps = pp_ctx.tile([P, QT], F32, tag="pctx")
                        den_ps = pp_den.tile([P, QT], F32, tag="pden")
                        q0 = c * QT
                        for g in range(ngrp):
                            grp_ps = pp_s.tile([P, GJ * QT], F32, tag="ps")
                            e_grp = exp_pool.tile([P, GJ * QT], BF16, tag="e")
                            offs = []
                            for s in range(GJ):
                                j = g * GJ + s
                                o = max(0, (j - 4 * c) * KC)
                                offs.append(o)
                                mm(grp_ps[:, s * QT + o:(s + 1) * QT],
                                   kv[:, j * KC:(j + 1) * KC],
                                   qv[:, q0 + o:q0 + QT],
                                   start=True, stop=True)
                            # exp over each contiguous written run (pads in
                            # diagonal groups are never written nor read)
                            runs = []
                            for s in range(GJ):
                                lo = s * QT + offs[s]
                                hi = (s + 1) * QT
                                if runs and runs[-1][1] == lo:
                                    runs[-1][1] = hi
                                else:
                                    runs.append([lo, hi])
                            for lo, hi in runs:
                                nc.scalar.activation(
                                    e_grp[:, lo:hi], grp_ps[:, lo:hi],
                                    AF.Exp, scale=SCALE)
                            for s in range(GJ):
                                j = g * GJ + s
                                o = offs[s]
                                if j >= 4 * c:      # diagonal: triangle mask
                                    nc.vector.tensor_mul(
                                        e_grp[:, s * QT + o:s * QT + o + P],
                                        e_grp[:, s * QT + o:s * QT + o + P],
                                        tri_sb[:])
                            for s in range(GJ):
                                j = g * GJ + s
                                o = offs[s]
                                esub = e_grp[:, s * QT + o:(s + 1) * QT]
                                mm(ctx_ps[:, o:],
                                   v_res[:, b * (T // P) + j,
                                         h * HD:(h + 1) * HD],
                                   esub,
                                   start=(j == 0), stop=(j == nj - 1),
                                   skip_group_check=True)
                                # denominator: ones-matmul partition-reduce,
                                # accumulated in PSUM across k-blocks
                                mm(den_ps[:, o:], ones_sb[:], esub,
                                   start=(j == 0), stop=(j == nj - 1),
                                   skip_group_check=True)
                        recip = rcp_pool.tile([P, QT], F32, tag="rcp")
                        nc.vector.reciprocal(recip[:], den_ps[:])
                        nc.vector.tensor_mul(
                            ctx_res[:, h, boff + q0:boff + q0 + QT],
                            ctx_ps[:], recip[:])

                for b in range(B):
                    for h in range(HPC):
                        do_pair(b, h)

            # ---------------- Phase 3: output projection ----------------
            with (
                tc.tile_pool(name="wo", bufs=1) as wo_pool,
                tc.tile_pool(name="st3", bufs=3) as st3_pool,
                tc.tile_pool(name="pp_o", bufs=2, space="PSUM") as pp_o,
            ):
                wo_sb = wo_pool.tile([P, HPC, D], BF16, tag="wo")
                nc.sync.dma_start(wo_sb[:], wo)
                NOD = D // QT
                for tb in range(TBLK):
                    ts2 = slice(tb * P, (tb + 1) * P)
                    ost = st3_pool.tile([P, D], BF16, tag="ost")
                    for od in range(NOD):
                        ods = slice(od * QT, (od + 1) * QT)
                        ps0 = pp_o.tile([P, QT], F32, tag="po")
                        mm(ps0[:], ctx_res[:, 0, ts2], wo_sb[:, 0, ods],
                           start=True, stop=False)
                        mm(ps0[:], ctx_res[:, 1, ts2], wo_sb[:, 1, ods],
                           start=False, stop=True)
                        if od % 2 == 0:
                            nc.vector.tensor_copy(ost[:, ods], ps0[:])
                        else:
                            nc.scalar.copy(ost[:, ods], ps0[:])
                    nc.sync.dma_start(out_p[ts2, :], ost[:])

    nc.compile()
    return nc


def _build_fused():
    """Software-pipelined variant: phase-1 projections of batch b+1 and
    phase-3 output blocks are interleaved (at emission level) with the
    attention of batch b, so the PE never idles on exp/copy latency."""
    import concourse.bass as bass  # noqa: F401
    import concourse.mybir as mybir
    from concourse import bacc
    import concourse.tile as tile

    F32 = mybir.dt.float32
    BF16 = mybir.dt.bfloat16
    AF = mybir.ActivationFunctionType

    SCALE = 1.0 / float(np.sqrt(HD))
    TPB = T // PT               # phase-1 tiles per batch (4)

    nc = bacc.Bacc("TRN2", target_bir_lowering=False, debug=False)

    xt = nc.dram_tensor("xt", [NPT, P, DK, PT], BF16, kind="ExternalInput").ap()
    wq = nc.dram_tensor("wq", [HPC, P, DK, P], BF16, kind="ExternalInput").ap()
    wk = nc.dram_tensor("wk", [HPC, P, DK, P], BF16, kind="ExternalInput").ap()
    wv = nc.dram_tensor("wv", [HPC, P, DK, P], BF16, kind="ExternalInput").ap()
    wo = nc.dram_tensor("wo", [P, HPC, D], BF16, kind="ExternalInput").ap()
    tri = nc.dram_tensor("tri", [P, P], BF16, kind="ExternalInput").ap()

    kT_out = nc.dram_tensor("kT_out", [DPC, NTOK], BF16, kind="ExternalOutput").ap()
    v_out = nc.dram_tensor("v_out", [NTOK, DPC], BF16, kind="ExternalOutput").ap()
    out_p = nc.dram_tensor("out_p", [NTOK, D], BF16, kind="ExternalOutput").ap()

    mm = nc.tensor.matmul

    def interleave(*gens):
        """Credit-based round-robin, preferring to alternate generators so
        single-buffered PSUM pools get their latency covered."""
        live = [[g, n] for g, n in gens if n > 0]
        credit = [0.0] * len(live)
        last = [None]
        while live:
            for i, it in enumerate(live):
                credit[i] += it[1]
            order = sorted(range(len(live)), key=lambda i: -credit[i])
            i = order[0]
            if live[i][0] is last[0] and len(order) > 1:
                i = order[1]
            credit[i] -= sum(it[1] for it in live)
            last[0] = live[i][0]
            try:
                next(live[i][0])
            except StopIteration:
                credit.pop(i)
                live.pop(i)

    with tile.TileContext(nc) as tc:
        with (
            tc.tile_pool(name="res", bufs=1) as res_pool,
            tc.tile_pool(name="cst", bufs=1) as cst_pool,
        ):
            q_res = res_pool.tile([P, HPC, NTOK], BF16, tag="q")
            k_res = res_pool.tile([P, HPC, NTOK], BF16, tag="k")
            v_res = res_pool.tile([P, TBLK, DPC], BF16, tag="v")
            ctx_res = res_pool.tile([P, HPC, NTOK], BF16, tag="ctx")
            tri_sb = cst_pool.tile([P, P], BF16, tag="tri")
            ones_f = cst_pool.tile([P, P], F32, tag="onesf")
            ones_sb = cst_pool.tile([P, P], BF16, tag="ones")
            wo_sb = cst_pool.tile([P, HPC, D], BF16, tag="wo")
            nc.sync.dma_start(tri_sb[:], tri)
            nc.vector.memset(ones_f[:], 1.0)
            nc.vector.tensor_copy(ones_sb[:], ones_f[:])

            with (
                tc.tile_pool(name="exp", bufs=3) as exp_pool,
                tc.tile_pool(name="rcp", bufs=1) as rcp_pool,
                tc.tile_pool(name="pp_s", bufs=1, space="PSUM") as pp_s,
                tc.tile_pool(name="pp_ctx", bufs=1, space="PSUM") as pp_ctx,
                tc.tile_pool(name="pp_den", bufs=1, space="PSUM") as pp_den,
            ):
                def p2_chunk(b, h, c):
                    """One yield per exp-group (2 k-blocks)."""
                    boff = b * T
                    qv = q_res[:, h, boff:boff + T]
                    kv = k_res[:, h, boff:boff + T]
                    if True:
                        nj = (c + 1) * (QT // KC)
                        ngrp = nj // GJ
                        ctx_ps = pp_ctx.tile([P, QT], F32, tag="pctx")
                        den_ps = pp_den.tile([P, QT], F32, tag="pden")
                        q0 = c * QT
                        for g in range(ngrp):
                            grp_ps = pp_s.tile([P, GJ * QT], F32, tag="ps")
                            e_grp = exp_pool.tile([P, GJ * QT], BF16, tag="e")
                            offs = []
                            for s in range(GJ):
                                j = g * GJ + s
                                o = max(0, (j - 4 * c) * KC)
                                offs.append(o)
                                mm(grp_ps[:, s * QT + o:(s + 1) * QT],
                                   kv[:, j * KC:(j + 1) * KC],
                                   qv[:, q0 + o:q0 + QT],
                                   start=True, stop=True)
                            runs = []
                            for s in range(GJ):
                                lo = s * QT + offs[s]
                                hi = (s + 1) * QT
                                if runs and runs[-1][1] == lo:
                                    runs[-1][1] = hi
                                else:
                                    runs.append([lo, hi])
                            for lo, hi in runs:
                                nc.scalar.activation(
                                    e_grp[:, lo:hi], grp_ps[:, lo:hi],
                                    AF.Exp, scale=SCALE)
                            for s in range(GJ):
                                j = g * GJ + s
                                o = offs[s]
                                if j >= 4 * c:
                                    nc.vector.tensor_mul(
                                        e_grp[:, s * QT + o:s * QT + o + P],
                                        e_grp[:, s * QT + o:s * QT + o + P],
                                        tri_sb[:])
                            for s in range(GJ):
                                j = g * GJ + s
                                o = offs[s]
                                esub = e_grp[:, s * QT + o:(s + 1) * QT]
                                mm(ctx_ps[:, o:],
                                   v_res[:, b * (T // P) + j,
                                         h * HD:(h + 1) * HD],
                                   esub,
                                   start=(j == 0), stop=(j == nj - 1),
                                   skip_group_check=True)
                                mm(den_ps[:, o:], ones_sb[:], esub,
                                   start=(j == 0), stop=(j == nj - 1),
                                   skip_group_check=True)
                            if g == ngrp - 1:
                                recip = rcp_pool.tile([P, QT], F32, tag="rcp")
                                nc.vector.reciprocal(recip[:], den_ps[:])
                                nc.vector.tensor_mul(
                                    ctx_res[:, h, boff + q0:boff + q0 + QT],
                                    ctx_ps[:], recip[:])
                            yield

                def p2_batch(b):
                    for h in range(HPC):
                        for c in range(NQC):
                            yield from p2_chunk(b, h, c)

                # ---- segment A: p1(b) interleaved with p2(b-1) ----
                with (
                    tc.tile_pool(name="wgt", bufs=1) as w_pool,
                    tc.tile_pool(name="xt", bufs=2) as xt_pool,
                    tc.tile_pool(name="pp_qk", bufs=1, space="PSUM") as pp_qk,
                    tc.tile_pool(name="pp_v", bufs=1, space="PSUM") as pp_v,
                ):
                    wq_sb = w_pool.tile([P, DK, DPC], BF16, tag="wq")
                    wk_sb = w_pool.tile([P, DK, DPC], BF16, tag="wk")
                    wv_sb = w_pool.tile([P, DK, DPC], BF16, tag="wv")
                    # split so the hc=0 chains start as soon as possible
                    for hc in range(HPC):
                        nc.sync.dma_start(
                            wq_sb[:, :, hc * P:(hc + 1) * P], wq[hc])

                    def load_xt(tb, split=False):
                        x_sb = xt_pool.tile([P, DK, PT], BF16, tag="x")
                        if split:
                            h1 = DK // 2
                            nc.sync.dma_start(x_sb[:, :h1, :], xt[tb, :, :h1, :])
                            nc.sync.dma_start(x_sb[:, h1:, :], xt[tb, :, h1:, :])
                        else:
                            nc.sync.dma_start(x_sb[:], xt[tb])
                        return x_sb

                    x0 = load_xt(0, split=True)
                    for hc in range(HPC):
                        nc.sync.dma_start(
                            wk_sb[:, :, hc * P:(hc + 1) * P], wk[hc])
                    for hc in range(HPC):
                        nc.sync.dma_start(
                            wv_sb[:, :, hc * P:(hc + 1) * P], wv[hc])
                    nc.sync.dma_start(wo_sb[:], wo)

                    def p1_units(b):
                        """One yield per 16-matmul chain (8 per tile)."""
                        for tloc in range(TPB):
                            tb = b * TPB + tloc
                            x_sb = x0 if tb == 0 else load_xt(tb)
                            ts = slice(tb * PT, (tb + 1) * PT)
                            for w_sb, dst in ((wq_sb, q_res), (wk_sb, k_res)):
                                for hc in range(HPC):
                                    ps = pp_qk.tile([P, PT], F32, tag="pqk")
                                    for dc in range(DK):
                                        mm(ps[:],
                                           w_sb[:, dc, hc * P:(hc + 1) * P],
                                           x_sb[:, dc, :],
                                           start=(dc == 0),
                                           stop=(dc == DK - 1))
                                    nc.vector.tensor_copy(dst[:, hc, ts], ps[:])
                                    if dst is k_res:
                                        nc.sync.dma_start(
                                            kT_out[hc * P:(hc + 1) * P, ts],
                                            k_res[:, hc, ts])
                                    yield
                            for sub in range(PT // P):
                                blk = tb * (PT // P) + sub
                                ps = pp_v.tile([P, DPC], F32, tag="pv")
                                for dc in range(DK):
                                    mm(ps[:],
                                       x_sb[:, dc, sub * P:(sub + 1) * P],
                                       wv_sb[:, dc, :],
                                       start=(dc == 0), stop=(dc == DK - 1))
                                nc.vector.tensor_copy(v_res[:, blk, :], ps[:])
                                nc.sync.dma_start(
                                    v_out[blk * P:(blk + 1) * P, :],
                                    v_res[:, blk, :])
                                yield

                    # prologue + first fused segment (no p3 yet)
                    for _ in p1_units(0):
                        pass
                    interleave((p1_units(1), 32), (p2_batch(0), 40))

                    with (
                        tc.tile_pool(name="st3", bufs=3) as st3_pool,
                        tc.tile_pool(name="pp_o", bufs=2, space="PSUM") as pp_o,
                    ):
                        NOD = D // QT

                        def p3_units(blocks):
                            """One yield per token block (8 matmuls)."""
                            for tb in blocks:
                                ts2 = slice(tb * P, (tb + 1) * P)
                                for half in range(2):
                                    ost = st3_pool.tile([P, D // 2], BF16,
                                                        tag="ost")
                                    for odl in range(NOD // 2):
                                        od = half * (NOD // 2) + odl
                                        ods = slice(od * QT, (od + 1) * QT)
                                        odl_s = slice(odl * QT,
                                                      (odl + 1) * QT)
                                        ps0 = pp_o.tile([P, QT], F32,
                                                        tag="po")
                                        mm(ps0[:], ctx_res[:, 0, ts2],
                                           wo_sb[:, 0, ods],
                                           start=True, stop=False)
                                        mm(ps0[:], ctx_res[:, 1, ts2],
                                           wo_sb[:, 1, ods],
                                           start=False, stop=True)
                                        nc.vector.tensor_copy(
                                            ost[:, odl_s], ps0[:])
                                    nc.sync.dma_start(
                                        out_p[ts2, half * (D // 2):
                                              (half + 1) * (D // 2)],
                                        ost[:])
                                yield

                        TB = T // P     # 16 token blocks per batch

                        def brange(b):
                            return range(b * TB, (b + 1) * TB)

                        # p1(b) || p2(b-1) || p3(b-2) for b = 2, 3
                        for b in (2, 3):
                            interleave((p1_units(b), 32),
                                       (p2_batch(b - 1), 40),
                                       (p3_units(brange(b - 2)), 16))

                        # tail: p2(3) chunk-major with p3(3) blocks inlined
                        # as their ctx becomes ready, p3(2) as filler
                        fill = p3_units(brange(2))

                        def p2_tail(b):
                            for c in range(NQC):
                                for h in range(HPC):
                                    yield from p2_chunk(b, h, c)
                                yield ("p3", c)

                        nfill = [16]
                        fcredit = [0.0]

                        def drain_fill(n):
                            fcredit[0] += n * (16.0 / 56.0)
                            while fcredit[0] >= 1.0 and nfill[0] > 0:
                                fcredit[0] -= 1.0
                                try:
                                    next(fill)
                                    nfill[0] -= 1
                                except StopIteration:
                                    nfill[0] = 0

                        b3 = B - 1
                        for unit in p2_tail(b3):
                            if isinstance(unit, tuple):
                                c = unit[1]
                                for _ in p3_units(
                                        range(b3 * TB + c * 4,
                                              b3 * TB + (c + 1) * 4)):
                                    drain_fill(1)
                            else:
                                drain_fill(1)
                        drain_fill(99)

    nc.compile()
    return nc


def _get_module():
    if "nc" not in _CACHE:
        if os.environ.get("BASS_FUSED", "1") == "1":
            _CACHE["nc"] = _build_fused()
        else:
            use_ldw = os.environ.get("BASS_USE_LDW", "0") == "1"
            _CACHE["nc"] = _build_module(use_ldw=use_ldw)
    return _CACHE["nc"]


def _make_tri():
    import ml_dtypes
    m = np.zeros((P, P), dtype=np.float32)
    for kk in range(P):
        m[kk, kk:] = 1.0
    return m.astype(ml_dtypes.bfloat16)


def _run(x, Wq, Wk, Wv, Wo, bo, trace=False):
    import ml_dtypes
    from concourse import bass_utils

    BF = ml_dtypes.bfloat16
    nc = _get_module()
    x = np.asarray(x, dtype=np.float32)
    # xt[tb, p, dk, t] = x[tb*PT + t, dk*P + p]
    xt = np.ascontiguousarray(
        x.reshape(NPT, PT, DK, P).transpose(0, 3, 2, 1)).astype(BF)
    tri = _make_tri()
    Wq = np.asarray(Wq, np.float32)
    Wk = np.asarray(Wk, np.float32)
    Wv = np.asarray(Wv, np.float32)
    Wo = np.asarray(Wo, np.float32)
    in_maps = []
    for c in range(N_CORES):
        sl = slice(c * DPC, (c + 1) * DPC)
        # w[p, dk, n] = W[c*DPC + n, dk*P + p]
        def wtile(W):
            # [hc, p, dk, d] with d = output dim within head
            return np.ascontiguousarray(
                W[sl, :].reshape(HPC, P, DK, P).transpose(0, 3, 2, 1)
            ).astype(BF)
        # wo[p, h, n] = Wo[n, c*DPC + h*P + p]
        wo_t = np.ascontiguousarray(
            Wo[:, sl].reshape(D, HPC, P).transpose(2, 1, 0)).astype(BF)
        in_maps.append({
            "xt": xt,
            "wq": wtile(Wq),
            "wk": wtile(Wk),
            "wv": wtile(Wv),
            "wo": wo_t,
            "tri": tri,
        })
    res = bass_utils.run_bass_kernel_spmd(
        nc, in_maps, core_ids=list(range(N_CORES)), trace=trace)

    out = np.zeros((NTOK, D), np.float32)
    k = np.empty((NTOK, D), np.float32)
    v = np.empty((NTOK, D), np.float32)
    for c, r in enumerate(res.results):
        sl = slice(c * DPC, (c + 1) * DPC)
        out += np.asarray(r["out_p"], dtype=np.float32)
        k[:, sl] = np.asarray(r["kT_out"], dtype=np.float32).T
        v[:, sl] = np.asarray(r["v_out"], dtype=np.float32)
    out += np.asarray(bo, np.float32)[None, :]
    outs = (out.reshape(B, T, D), k.reshape(B, T, D), v.reshape(B, T, D))
    return outs, res


def kernel(x, Wq, Wk, Wv, Wo, bo):
    outs, _ = _run(x, Wq, Wk, Wv, Wo, bo, trace=False)
    return outs


# revision 19
# speedup vs baseline: 1.3551x; 1.0169x over previous
"""Trainium2 Bass kernel for nn_MultiHeadAttention (B=4, T=2048, D=2048, H=16).

Sharding: tensor-parallel over heads. Each of 8 NeuronCores owns 2 heads
(256 of the 2048 Q/K/V dims). All matmul data is bf16 (1 cycle/row on the
PE at any width; rel-err budget 2e-2 leaves plenty of room).

Per core:
  phase 1: qT/kT projections in transposed layout [head_dim, tokens] and v
           in [tokens, head_dim] layout, streaming host-pretiled x-tiles.
           q/k/v stay RESIDENT in SBUF (bf16, 12MB); k/v also DMA out as
           bf16 external outputs (host casts to f32).
  phase 2: per (batch, head): causal attention at 128-row k-block
           granularity. Scores matmuls write PSUM groups of 2 k-blocks;
           one big exp activation per group (amortizes the ACT engine's
           352-cycle instruction overhead); 128x128 triangle mask on
           diagonal blocks only; denominator accumulated per-partition by
           the DVE and partition-reduced by one ones-matmul per q-chunk;
           AV accumulates into PSUM, then ctx = ctx_ps * recip(den) (DVE)
           into a resident bf16 ctx buffer.
  phase 3: out_partial[tok, :] = sum_h ctx_h.T @ WoT_h, written bf16.
Host: partials summed across cores in f32; k/v slices concatenated.
"""

import os
import sys

import numpy as np

for _p in ("/opt/trn_rl_repo",):
    if _p not in sys.path and os.path.isdir(_p):
        sys.path.insert(0, _p)

B, T, D, H = 4, 2048, 2048, 16
HD = 128
N_CORES = 8
HPC = H // N_CORES          # heads per core
DPC = HPC * HD              # q/k/v dims per core
NTOK = B * T

P = 128
QT = 512                    # q-chunk width (PSUM bank = 512 f32)
KC = 128                    # k-block granularity (= partition dim)
PT = 512                    # phase-1 token tile
DK = D // P                 # 16 contraction chunks
NPT = NTOK // PT            # 16 phase-1 tiles
NQC = T // QT               # 4 q-chunks per batch
TBLK = NTOK // P            # 64 phase-3 token blocks
GJ = 2                      # k-blocks per exp group

_CACHE = {}


def _build_module(use_ldw=False):
    import concourse.bass as bass  # noqa: F401
    import concourse.mybir as mybir
    from concourse import bacc
    import concourse.tile as tile

    F32 = mybir.dt.float32
    F32R = mybir.dt.float32r
    BF16 = mybir.dt.bfloat16
    AF = mybir.ActivationFunctionType

    SCALE = 1.0 / float(np.sqrt(HD))

    nc = bacc.Bacc("TRN2", target_bir_lowering=False, debug=False)

    # host-pretiled inputs (all bf16, partition-major contiguous)
    xt = nc.dram_tensor("xt", [NPT, P, DK, PT], BF16, kind="ExternalInput").ap()
    wq = nc.dram_tensor("wq", [P, DK, DPC], BF16, kind="ExternalInput").ap()
    wk = nc.dram_tensor("wk", [P, DK, DPC], BF16, kind="ExternalInput").ap()
    wv = nc.dram_tensor("wv", [P, DK, DPC], BF16, kind="ExternalInput").ap()
    wo = nc.dram_tensor("wo", [P, HPC, D], BF16, kind="ExternalInput").ap()
    tri = nc.dram_tensor("tri", [P, P], BF16, kind="ExternalInput").ap()

    kT_out = nc.dram_tensor("kT_out", [DPC, NTOK], BF16, kind="ExternalOutput").ap()
    v_out = nc.dram_tensor("v_out", [NTOK, DPC], BF16, kind="ExternalOutput").ap()
    out_p = nc.dram_tensor("out_p", [NTOK, D], BF16, kind="ExternalOutput").ap()

    def mm(out, lhsT, rhs, **kw):
        if use_ldw and lhsT.dtype not in (F32, F32R):
            nc.tensor.ldweights(lhsT)
        nc.tensor.matmul(out, lhsT, rhs, **kw)

    with tile.TileContext(nc) as tc:
        with (
            tc.tile_pool(name="res", bufs=1) as res_pool,
            tc.tile_pool(name="cst", bufs=1) as cst_pool,
        ):
            q_res = res_pool.tile([P, HPC, NTOK], BF16, tag="q")
            k_res = res_pool.tile([P, HPC, NTOK], BF16, tag="k")
            v_res = res_pool.tile([P, TBLK, DPC], BF16, tag="v")
            ctx_res = res_pool.tile([P, HPC, NTOK], BF16, tag="ctx")
            tri_sb = cst_pool.tile([P, P], BF16, tag="tri")
            ones_f = cst_pool.tile([P, P], F32, tag="onesf")
            ones_sb = cst_pool.tile([P, P], BF16, tag="ones")
            nc.sync.dma_start(tri_sb[:], tri)
            nc.vector.memset(ones_f[:], 1.0)
            nc.vector.tensor_copy(ones_sb[:], ones_f[:])

            # ---------------- Phase 1: projections ----------------
            with (
                tc.tile_pool(name="wgt", bufs=1) as w_pool,
                tc.tile_pool(name="xt", bufs=2) as xt_pool,
                tc.tile_pool(name="pp_qk", bufs=2, space="PSUM") as pp_qk,
                tc.tile_pool(name="pp_v", bufs=2, space="PSUM") as pp_v,
            ):
                wq_sb = w_pool.tile([P, DK, DPC], BF16, tag="wq")
                wk_sb = w_pool.tile([P, DK, DPC], BF16, tag="wk")
                wv_sb = w_pool.tile([P, DK, DPC], BF16, tag="wv")
                nc.sync.dma_start(wq_sb[:], wq)

                def load_xt(tb):
                    x_sb = xt_pool.tile([P, DK, PT], BF16, tag="x")
                    nc.sync.dma_start(x_sb[:], xt[tb])
                    return x_sb

                x0 = load_xt(0)
                nc.sync.dma_start(wk_sb[:], wk)
                nc.sync.dma_start(wv_sb[:], wv)

                for tb in range(NPT):
                    x_sb = x0 if tb == 0 else load_xt(tb)
                    ts = slice(tb * PT, (tb + 1) * PT)

                    for w_sb, dst in ((wq_sb, q_res), (wk_sb, k_res)):
                        for hc in range(HPC):
                            ps = pp_qk.tile([P, PT], F32, tag="pqk")
                            for dc in range(DK):
                                mm(ps[:],
                                   w_sb[:, dc, hc * P:(hc + 1) * P],
                                   x_sb[:, dc, :],
                                   start=(dc == 0), stop=(dc == DK - 1))
                            nc.vector.tensor_copy(dst[:, hc, ts], ps[:])
                            if dst is k_res:
                                nc.sync.dma_start(
                                    kT_out[hc * P:(hc + 1) * P, ts],
                                    k_res[:, hc, ts])

                    for sub in range(PT // P):
                        blk = tb * (PT // P) + sub
                        ps = pp_v.tile([P, DPC], F32, tag="pv")
                        for dc in range(DK):
                            mm(ps[:],
                               x_sb[:, dc, sub * P:(sub + 1) * P],
                               wv_sb[:, dc, :],
                               start=(dc == 0), stop=(dc == DK - 1))
                        nc.vector.tensor_copy(v_res[:, blk, :], ps[:])
                        nc.sync.dma_start(
                            v_out[blk * P:(blk + 1) * P, :], v_res[:, blk, :])

            # ---------------- Phase 2: attention ----------------
            with (
                tc.tile_pool(name="exp", bufs=4) as exp_pool,
                tc.tile_pool(name="rcp", bufs=2) as rcp_pool,
                tc.tile_pool(name="pp_s", bufs=2, space="PSUM") as pp_s,
                tc.tile_pool(name="pp_ctx", bufs=2, space="PSUM") as pp_ctx,
                tc.tile_pool(name="pp_den", bufs=2, space="PSUM") as pp_den,
            ):
                def do_pair(b, h):
                    boff = b * T
                    qv = q_res[:, h, boff:boff + T]
                    kv = k_res[:, h, boff:boff + T]
                    for c in range(NQC):
                        nj = (c + 1) * (QT // KC)       # active k-blocks
                        ngrp = nj // GJ
                        ctx_ps = pp_ctx.tile([P, QT], F32, tag="pctx")
                        den_ps = pp_den.tile([P, QT], F32, tag="pden")
                        q0 = c * QT
                        for g in range(ngrp):
                            grp_ps = pp_s.tile([P, GJ * QT], F32, tag="ps")
                            e_grp = exp_pool.tile([P, GJ * QT], BF16, tag="e")
                            offs = []
                            for s in range(GJ):
                                j = g * GJ + s
                                o = max(0, (j - 4 * c) * KC)
                                offs.append(o)
                                mm(grp_ps[:, s * QT + o:(s + 1) * QT],
                                   kv[:, j * KC:(j + 1) * KC],
                                   qv[:, q0 + o:q0 + QT],
                                   start=True, stop=True)
                            # exp over each contiguous written run (pads in
                            # diagonal groups are never written nor read)
                            runs = []
                            for s in range(GJ):
                                lo = s * QT + offs[s]
                                hi = (s + 1) * QT
                                if runs and runs[-1][1] == lo:
                                    runs[-1][1] = hi
                                else:
                                    runs.append([lo, hi])
                            for lo, hi in runs:
                                nc.scalar.activation(
                                    e_grp[:, lo:hi], grp_ps[:, lo:hi],
                                    AF.Exp, scale=SCALE)
                            for s in range(GJ):
                                j = g * GJ + s
                                o = offs[s]
                                if j >= 4 * c:      # diagonal: triangle mask
                                    nc.vector.tensor_mul(
                                        e_grp[:, s * QT + o:s * QT + o + P],
                                        e_grp[:, s * QT + o:s * QT + o + P],
                                        tri_sb[:])
                            for s in range(GJ):
                                j = g * GJ + s
                                o = offs[s]
                                esub = e_grp[:, s * QT + o:(s + 1) * QT]
                                mm(ctx_ps[:, o:],
                                   v_res[:, b * (T // P) + j,
                                         h * HD:(h + 1) * HD],
                                   esub,
                                   start=(j == 0), stop=(j == nj - 1),
                                   skip_group_check=True)
                                # denominator: ones-matmul partition-reduce,
                                # accumulated in PSUM across k-blocks
                                mm(den_ps[:, o:], ones_sb[:], esub,
                                   start=(j == 0), stop=(j == nj - 1),
                                   skip_group_check=True)
                        recip = rcp_pool.tile([P, QT], F32, tag="rcp")
                        nc.vector.reciprocal(recip[:], den_ps[:])
                        nc.vector.tensor_mul(
                            ctx_res[:, h, boff + q0:boff + q0 + QT],
                            ctx_ps[:], recip[:])

                for b in range(B):
                    for h in range(HPC):
                        do_pair(b, h)

            # ---------------- Phase 3: output projection ----------------
            with (
                tc.tile_pool(name="wo", bufs=1) as wo_pool,
                tc.tile_pool(name="st3", bufs=3) as st3_pool,
                tc.tile_pool(name="pp_o", bufs=2, space="PSUM") as pp_o,
            ):
                wo_sb = wo_pool.tile([P, HPC, D], BF16, tag="wo")
                nc.sync.dma_start(wo_sb[:], wo)
                NOD = D // QT
                for tb in range(TBLK):
                    ts2 = slice(tb * P, (tb + 1) * P)
                    ost = st3_pool.tile([P, D], BF16, tag="ost")
                    for od in range(NOD):
                        ods = slice(od * QT, (od + 1) * QT)
                        ps0 = pp_o.tile([P, QT], F32, tag="po")
                        mm(ps0[:], ctx_res[:, 0, ts2], wo_sb[:, 0, ods],
                           start=True, stop=False)
                        mm(ps0[:], ctx_res[:, 1, ts2], wo_sb[:, 1, ods],
                           start=False, stop=True)
                        if od % 2 == 0:
                            nc.vector.tensor_copy(ost[:, ods], ps0[:])
                        else:
                            nc.scalar.copy(ost[:, ods], ps0[:])
                    nc.sync.dma_start(out_p[ts2, :], ost[:])

    nc.compile()
    return nc


def _build_fused():
    """Software-pipelined variant: phase-1 projections of batch b+1 and
    phase-3 output blocks are interleaved (at emission level) with the
    attention of batch b, so the PE never idles on exp/copy latency."""
    import concourse.bass as bass  # noqa: F401
    import concourse.mybir as mybir
    from concourse import bacc
    import concourse.tile as tile

    F32 = mybir.dt.float32
    BF16 = mybir.dt.bfloat16
    AF = mybir.ActivationFunctionType

    SCALE = 1.0 / float(np.sqrt(HD))
    TPB = T // PT               # phase-1 tiles per batch (4)

    nc = bacc.Bacc("TRN2", target_bir_lowering=False, debug=False)

    xt = nc.dram_tensor("xt", [NPT, P, DK, PT], BF16, kind="ExternalInput").ap()
    wq = nc.dram_tensor("wq", [HPC, P, DK, P], BF16, kind="ExternalInput").ap()
    wk = nc.dram_tensor("wk", [HPC, P, DK, P], BF16, kind="ExternalInput").ap()
    wv = nc.dram_tensor("wv", [HPC, P, DK, P], BF16, kind="ExternalInput").ap()
    wo = nc.dram_tensor("wo", [P, HPC, D], BF16, kind="ExternalInput").ap()
    tri = nc.dram_tensor("tri", [P, P], BF16, kind="ExternalInput").ap()

    kT_out = nc.dram_tensor("kT_out", [DPC, NTOK], BF16, kind="ExternalOutput").ap()
    v_out = nc.dram_tensor("v_out", [NTOK, DPC], BF16, kind="ExternalOutput").ap()
    out_p = nc.dram_tensor("out_p", [NTOK, D], BF16, kind="ExternalOutput").ap()

    mm = nc.tensor.matmul

    def interleave(*gens):
        """Credit-based round-robin, preferring to alternate generators so
        single-buffered PSUM pools get their latency covered."""
        live = [[g, n] for g, n in gens if n > 0]
        credit = [0.0] * len(live)
        last = [None]
        while live:
            for i, it in enumerate(live):
                credit[i] += it[1]
            order = sorted(range(len(live)), key=lambda i: -credit[i])
            i = order[0]
            if live[i][0] is last[0] and len(order) > 1:
                i = order[1]
            credit[i] -= sum(it[1] for it in live)
            last[0] = live[i][0]
            try:
                next(live[i][0])
            except StopIteration:
                credit.pop(i)
                live.pop(i)

    with tile.TileContext(nc) as tc:
        with (
            tc.tile_pool(name="res", bufs=1) as res_pool,
            tc.tile_pool(name="cst", bufs=1) as cst_pool,
        ):
            q_res = res_pool.tile([P, HPC, NTOK], BF16, tag="q")
            k_res = res_pool.tile([P, HPC, NTOK], BF16, tag="k")
            v_res = res_pool.tile([P, TBLK, DPC], BF16, tag="v")
            ctx_res = res_pool.tile([P, HPC, NTOK], BF16, tag="ctx")
            tri_sb = cst_pool.tile([P, P], BF16, tag="tri")
            ones_f = cst_pool.tile([P, P], F32, tag="onesf")
            ones_sb = cst_pool.tile([P, P], BF16, tag="ones")
            wo_sb = cst_pool.tile([P, HPC, D], BF16, tag="wo")
            nc.sync.dma_start(tri_sb[:], tri)
            nc.vector.memset(ones_f[:], 1.0)
            nc.vector.tensor_copy(ones_sb[:], ones_f[:])

            with (
                tc.tile_pool(name="exp", bufs=3) as exp_pool,
                tc.tile_pool(name="rcp", bufs=1) as rcp_pool,
                tc.tile_pool(name="pp_s", bufs=1, space="PSUM") as pp_s,
                tc.tile_pool(name="pp_ctx", bufs=1, space="PSUM") as pp_ctx,
                tc.tile_pool(name="pp_den", bufs=1, space="PSUM") as pp_den,
            ):
                def p2_chunk(b, h, c):
                    """One yield per exp-group (2 k-blocks)."""
                    boff = b * T
                    qv = q_res[:, h, boff:boff + T]
                    kv = k_res[:, h, boff:boff + T]
                    if True:
                        nj = (c + 1) * (QT // KC)
                        ngrp = nj // GJ
                        ctx_ps = pp_ctx.tile([P, QT], F32, tag="pctx")
                        den_ps = pp_den.tile([P, QT], F32, tag="pden")
                        q0 = c * QT
                        for g in range(ngrp):
                            grp_ps = pp_s.tile([P, GJ * QT], F32, tag="ps")
                            e_grp = exp_pool.tile([P, GJ * QT], BF16, tag="e")
                            offs = []
                            for s in range(GJ):
                                j = g * GJ + s
                                o = max(0, (j - 4 * c) * KC)
                                offs.append(o)
                                mm(grp_ps[:, s * QT + o:(s + 1) * QT],
                                   kv[:, j * KC:(j + 1) * KC],
                                   qv[:, q0 + o:q0 + QT],
                                   start=True, stop=True)
                            runs = []
                            for s in range(GJ):
                                lo = s * QT + offs[s]
                                hi = (s + 1) * QT
                                if runs and runs[-1][1] == lo:
                                    runs[-1][1] = hi
                                else:
                                    runs.append([lo, hi])
                            for lo, hi in runs:
                                nc.scalar.activation(
                                    e_grp[:, lo:hi], grp_ps[:, lo:hi],
                                    AF.Exp, scale=SCALE)
                            for s in range(GJ):
                                j = g * GJ + s
                                o = offs[s]
                                if j >= 4 * c:
                                    nc.vector.tensor_mul(
                                        e_grp[:, s * QT + o:s * QT + o + P],
                                        e_grp[:, s * QT + o:s * QT + o + P],
                                        tri_sb[:])
                            for s in range(GJ):
                                j = g * GJ + s
                                o = offs[s]
                                esub = e_grp[:, s * QT + o:(s + 1) * QT]
                                mm(ctx_ps[:, o:],
                                   v_res[:, b * (T // P) + j,
                                         h * HD:(h + 1) * HD],
                                   esub,
                                   start=(j == 0), stop=(j == nj - 1),
                                   skip_group_check=True)
                                mm(den_ps[:, o:], ones_sb[:], esub,
                                   start=(j == 0), stop=(j == nj - 1),
                                   skip_group_check=True)
                            if g == ngrp - 1:
                                recip = rcp_pool.tile([P, QT], F32, tag="rcp")
                                nc.vector.reciprocal(recip[:], den_ps[:])
                                nc.vector.tensor_mul(
                                    ctx_res[:, h, boff + q0:boff + q0 + QT],
                                    ctx_ps[:], recip[:])
                            yield

                def p2_batch(b):
                    for h in range(HPC):
                        for c in range(NQC):
                            yield from p2_chunk(b, h, c)

                # ---- segment A: p1(b) interleaved with p2(b-1) ----
                with (
                    tc.tile_pool(name="wgt", bufs=1) as w_pool,
                    tc.tile_pool(name="xt", bufs=2) as xt_pool,
                    tc.tile_pool(name="pp_qk", bufs=1, space="PSUM") as pp_qk,
                    tc.tile_pool(name="pp_v", bufs=1, space="PSUM") as pp_v,
                ):
                    wq_sb = w_pool.tile([P, DK, DPC], BF16, tag="wq")
                    wk_sb = w_pool.tile([P, DK, DPC], BF16, tag="wk")
                    wv_sb = w_pool.tile([P, DK, DPC], BF16, tag="wv")
                    # split so the hc=0 chains start as soon as possible
                    for hc in range(HPC):
                        nc.sync.dma_start(
                            wq_sb[:, :, hc * P:(hc + 1) * P], wq[hc])

                    def load_xt(tb, split=False):
                        x_sb = xt_pool.tile([P, DK, PT], BF16, tag="x")
                        if split:
                            h1 = DK // 2
                            nc.sync.dma_start(x_sb[:, :h1, :], xt[tb, :, :h1, :])
                            nc.sync.dma_start(x_sb[:, h1:, :], xt[tb, :, h1:, :])
                        else:
                            nc.sync.dma_start(x_sb[:], xt[tb])
                        return x_sb

                    x0 = load_xt(0, split=True)
                    for hc in range(HPC):
                        nc.sync.dma_start(
                            wk_sb[:, :, hc * P:(hc + 1) * P], wk[hc])
                    for hc in range(HPC):
                        nc.sync.dma_start(
                            wv_sb[:, :, hc * P:(hc + 1) * P], wv[hc])
                    nc.sync.dma_start(wo_sb[:], wo)

                    def p1_units(b):
                        """One yield per 16-matmul chain (8 per tile)."""
                        for tloc in range(TPB):
                            tb = b * TPB + tloc
                            x_sb = x0 if tb == 0 else load_xt(tb)
                            ts = slice(tb * PT, (tb + 1) * PT)
                            for w_sb, dst in ((wq_sb, q_res), (wk_sb, k_res)):
                                for hc in range(HPC):
                                    ps = pp_qk.tile([P, PT], F32, tag="pqk")
                                    for dc in range(DK):
                                        mm(ps[:],
                                           w_sb[:, dc, hc * P:(hc + 1) * P],
                                           x_sb[:, dc, :],
                                           start=(dc == 0),
                                           stop=(dc == DK - 1))
                                    if hc == 0:
                                        nc.vector.tensor_copy(
                                            dst[:, hc, ts], ps[:])
                                    else:
                                        nc.scalar.copy(dst[:, hc, ts], ps[:])
                                    if dst is k_res:
                                        nc.sync.dma_start(
                                            kT_out[hc * P:(hc + 1) * P, ts],
                                            k_res[:, hc, ts])
                                    yield
                            for sub in range(PT // P):
                                blk = tb * (PT // P) + sub
                                ps = pp_v.tile([P, DPC], F32, tag="pv")
                                for dc in range(DK):
                                    mm(ps[:],
                                       x_sb[:, dc, sub * P:(sub + 1) * P],
                                       wv_sb[:, dc, :],
                                       start=(dc == 0), stop=(dc == DK - 1))
                                nc.scalar.copy(v_res[:, blk, :], ps[:])
                                nc.sync.dma_start(
                                    v_out[blk * P:(blk + 1) * P, :],
                                    v_res[:, blk, :])
                                yield

                    # prologue + first fused segment (no p3 yet)
                    for _ in p1_units(0):
                        pass
                    interleave((p1_units(1), 32), (p2_batch(0), 40))

                    with (
                        tc.tile_pool(name="st3", bufs=3) as st3_pool,
                        tc.tile_pool(name="pp_o", bufs=2, space="PSUM") as pp_o,
                    ):
                        NOD = D // QT

                        def p3_units(blocks):
                            """One yield per token block (8 matmuls)."""
                            for tb in blocks:
                                ts2 = slice(tb * P, (tb + 1) * P)
                                for half in range(2):
                                    ost = st3_pool.tile([P, D // 2], BF16,
                                                        tag="ost")
                                    for odl in range(NOD // 2):
                                        od = half * (NOD // 2) + odl
                                        ods = slice(od * QT, (od + 1) * QT)
                                        odl_s = slice(odl * QT,
                                                      (odl + 1) * QT)
                                        ps0 = pp_o.tile([P, QT], F32,
                                                        tag="po")
                                        mm(ps0[:], ctx_res[:, 0, ts2],
                                           wo_sb[:, 0, ods],
                                           start=True, stop=False)
                                        mm(ps0[:], ctx_res[:, 1, ts2],
                                           wo_sb[:, 1, ods],
                                           start=False, stop=True)
                                        if od % 2 == 0:
                                            nc.vector.tensor_copy(
                                                ost[:, odl_s], ps0[:])
                                        else:
                                            nc.scalar.copy(
                                                ost[:, odl_s], ps0[:])
                                    nc.sync.dma_start(
                                        out_p[ts2, half * (D // 2):
                                              (half + 1) * (D // 2)],
                                        ost[:])
                                yield

                        TB = T // P     # 16 token blocks per batch

                        def brange(b):
                            return range(b * TB, (b + 1) * TB)

                        # p1(b) || p2(b-1) || p3(b-2) for b = 2, 3
                        for b in (2, 3):
                            interleave((p1_units(b), 32),
                                       (p2_batch(b - 1), 40),
                                       (p3_units(brange(b - 2)), 16))

                        # tail: p2(3) chunk-major with p3(3) blocks inlined
                        # as their ctx becomes ready, p3(2) as filler
                        fill = p3_units(brange(2))

                        def p2_tail(b):
                            for c in range(NQC):
                                for h in range(HPC):
                                    yield from p2_chunk(b, h, c)
                                yield ("p3", c)

                        nfill = [16]
                        fcredit = [0.0]

                        def drain_fill(n):
                            fcredit[0] += n * (16.0 / 56.0)
                            while fcredit[0] >= 1.0 and nfill[0] > 0:
                                fcredit[0] -= 1.0
                                try:
                                    next(fill)
                                    nfill[0] -= 1
                                except StopIteration:
                                    nfill[0] = 0

                        b3 = B - 1
                        for unit in p2_tail(b3):
                            if isinstance(unit, tuple):
                                c = unit[1]
                                for _ in p3_units(
                                        range(b3 * TB + c * 4,
                                              b3 * TB + (c + 1) * 4)):
                                    drain_fill(1)
                            else:
                                drain_fill(1)
                        drain_fill(99)

    nc.compile()
    return nc


def _get_module():
    if "nc" not in _CACHE:
        if os.environ.get("BASS_FUSED", "1") == "1":
            _CACHE["nc"] = _build_fused()
        else:
            use_ldw = os.environ.get("BASS_USE_LDW", "0") == "1"
            _CACHE["nc"] = _build_module(use_ldw=use_ldw)
    return _CACHE["nc"]


def _make_tri():
    import ml_dtypes
    m = np.zeros((P, P), dtype=np.float32)
    for kk in range(P):
        m[kk, kk:] = 1.0
    return m.astype(ml_dtypes.bfloat16)


def _run(x, Wq, Wk, Wv, Wo, bo, trace=False):
    import ml_dtypes
    from concourse import bass_utils

    BF = ml_dtypes.bfloat16
    nc = _get_module()
    x = np.asarray(x, dtype=np.float32)
    # xt[tb, p, dk, t] = x[tb*PT + t, dk*P + p]
    xt = np.ascontiguousarray(
        x.reshape(NPT, PT, DK, P).transpose(0, 3, 2, 1)).astype(BF)
    tri = _make_tri()
    Wq = np.asarray(Wq, np.float32)
    Wk = np.asarray(Wk, np.float32)
    Wv = np.asarray(Wv, np.float32)
    Wo = np.asarray(Wo, np.float32)
    in_maps = []
    for c in range(N_CORES):
        sl = slice(c * DPC, (c + 1) * DPC)
        # w[p, dk, n] = W[c*DPC + n, dk*P + p]
        def wtile(W):
            # [hc, p, dk, d] with d = output dim within head
            return np.ascontiguousarray(
                W[sl, :].reshape(HPC, P, DK, P).transpose(0, 3, 2, 1)
            ).astype(BF)
        # wo[p, h, n] = Wo[n, c*DPC + h*P + p]
        wo_t = np.ascontiguousarray(
            Wo[:, sl].reshape(D, HPC, P).transpose(2, 1, 0)).astype(BF)
        in_maps.append({
            "xt": xt,
            "wq": wtile(Wq),
            "wk": wtile(Wk),
            "wv": wtile(Wv),
            "wo": wo_t,
            "tri": tri,
        })
    res = bass_utils.run_bass_kernel_spmd(
        nc, in_maps, core_ids=list(range(N_CORES)), trace=trace)

    out = np.zeros((NTOK, D), np.float32)
    k = np.empty((NTOK, D), np.float32)
    v = np.empty((NTOK, D), np.float32)
    for c, r in enumerate(res.results):
        sl = slice(c * DPC, (c + 1) * DPC)
        out += np.asarray(r["out_p"], dtype=np.float32)
        k[:, sl] = np.asarray(r["kT_out"], dtype=np.float32).T
        v[:, sl] = np.asarray(r["v_out"], dtype=np.float32)
    out += np.asarray(bo, np.float32)[None, :]
    outs = (out.reshape(B, T, D), k.reshape(B, T, D), v.reshape(B, T, D))
    return outs, res


def kernel(x, Wq, Wk, Wv, Wo, bo):
    outs, _ = _run(x, Wq, Wk, Wv, Wo, bo, trace=False)
    return outs
